# revision 1
# baseline (speedup 1.0000x reference)
"""Trainium2 kernel for CollectNeighbourAverageAndMax (gnn message passing).

out[v] = concat(mean_k x[idxs[v,k]], max_k x[idxs[v,k]]),  V=100000, K=32, F=64.

Sharding: vertices split across 8 NeuronCores (one SPMD program); x is
replicated per core so the irregular gather stays core-local.

Gather primitive: nc.gpsimd.dma_gather (ANT extended instruction) — int16
indices, 256B-stride window of <=32768 rows. x is repacked on the host into
x_aug: four 25000-row blocks (window = block), each biased by +BIAS and
followed by one zero row (the padding target). Every neighbour index becomes
(window, local) with local <= 25000, which fits int16.

Per vertex the neighbour list is grouped by window (order is irrelevant for
mean/max). Vertices are globally sorted by their per-window count profile so
that the 1024 vertices sharing tile slot t across the 8 cores have nearly
identical profiles; the per-tile per-window column counts W[t][s] (= max
count in that tile row) are baked into the compiled program, and short rows
are padded with the window's zero row. Since the bias makes all real values
positive, padding zeros never win the max, and they add 0 to the sum:
   mean = sum/K - BIAS,   max = max' - BIAS.

The tree: per super-tile of G tiles, one dma_gather per window (chunked to
<= NI_CAP indices per call) fills an SBUF tile whose columns are grouped by
(window, tile); per tile, strided tensor_reduce produces per-window partial
sum/max, combined across windows on DVE, finalized on ACT, stored per tile.
"""
import sys
import types
from contextlib import ExitStack

import numpy as np

V, K, F = 100000, 32, 64
N_CORES = 8
P = 128
NW = 4
BLK = 25000
AUGB = BLK + 1          # window span incl. the zero pad row (local 25000)
VAUG = NW * AUGB        # 100004
PADLOC = BLK
BIAS = 16.0
NB = (V + P - 1) // P + (-((V + P - 1) // P)) % N_CORES  # 784 blocks (=8*98)
NT = NB // N_CORES      # 98 tiles per core
VS_PAD = NT * P         # 12544 padded vertices per core
G = 4                   # tiles per super-tile
NI_CAP = 1024           # max indices per dma_gather call (multiple of 128; >1024 faults)


def _install_ntff_hook():
    try:
        import antenv

        if "antenv.axon_hooks" not in sys.modules:
            mod = types.ModuleType("antenv.axon_hooks")
            mod._hook = None
            mod.set_axon_ntff_profile_hook = lambda h: setattr(mod, "_hook", h)
            mod.get_axon_ntff_profile_hook = lambda: mod._hook
            sys.modules["antenv.axon_hooks"] = mod
            antenv.axon_hooks = mod
        if sys.modules["antenv.axon_hooks"]._hook is None:
            from trn_agent_boot.trn_boot import _ntff_profile_via_ctypes

            hook = _ntff_profile_via_ctypes("/opt/axon/libaxon_pjrt.so")
            sys.modules["antenv.axon_hooks"].set_axon_ntff_profile_hook(hook)
    except Exception:
        pass


def _plan(idxs):
    """Host-side plan from the index matrix.

    Returns (order [NB*P] int64 vertex ids w/ -1 padding,
             W [NT, NW] per-tile window widths,
             locs [V, K] window-grouped local indices,
             bnds [V, NW+1] per-vertex window boundaries in locs)."""
    win = idxs // BLK                     # [V, K]
    loc = (idxs % BLK).astype(np.int64)
    ordk = np.argsort(win, axis=1, kind="stable")
    locs = np.take_along_axis(loc, ordk, axis=1)          # grouped by window
    cnt = np.stack([(win == s).sum(1) for s in range(NW)], 1)  # [V, NW]
    bnds = np.zeros((V, NW + 1), dtype=np.int64)
    np.cumsum(cnt, axis=1, out=bnds[:, 1:])
    order = np.lexsort((cnt[:, 2], cnt[:, 1], cnt[:, 0]))  # [V]
    order = np.concatenate([order, np.full(NB * P - V, -1, dtype=np.int64)])
    # W[t, s] = max count over the 1024 vertices in blocks 8t..8t+7
    grp = order.reshape(NT, N_CORES * P)           # tile t rows across cores
    cnt_pad = np.concatenate([cnt, np.zeros((1, NW), dtype=np.int64)])  # -1 -> 0
    W = cnt_pad[grp].max(axis=1)                   # [NT, NW]
    return order, W, locs, bnds


def _call_schedule(W):
    """Static per-super-tile call schedule from W.

    Returns list over super-tiles of dicts:
      tiles: [t0..), per-tile col offsets A[j][s] (col units), Ctot,
      calls: list of (s, [(tile_idx, col_lo, col_hi)...], n_idx, col_base)
    Chunked so each call has <= NI_CAP indices."""
    supers = []
    for u in range((NT + G - 1) // G):
        tiles = list(range(u * G, min((u + 1) * G, NT)))
        colbase = 0
        A = {}
        calls = []
        for s in range(NW):
            # contiguous column run for window s across this super's tiles
            run = []
            for j in tiles:
                w = int(W[j, s])
                A[(j, s)] = colbase
                if w:
                    run.append((j, 0, w))
                    colbase += w
            # chunk the run into <= NI_CAP/P columns per call
            maxcols = NI_CAP // P
            flat = []
            for j, lo, hi in run:
                for c in range(lo, hi):
                    flat.append((j, c))
            for i in range(0, len(flat), maxcols):
                chunk = flat[i : i + maxcols]
                calls.append((s, chunk, len(chunk) * P))
        supers.append({"tiles": tiles, "A": A, "Ctot": colbase, "calls": calls})
    return supers


def _pack_idx16(core, order, W, locs, bnds, supers):
    """Build this core's flat int16 index stream. Per super-tile: the calls'
    wrapped [128, iw_call] blocks are concatenated along the free dim into
    [128, iw_sup], then raveled partition-major ("(p w)")."""
    per_super_flat = []
    for u, sup in enumerate(supers):
        blocks = []
        for s, chunk, ni in sup["calls"]:
            cols = []
            for j, c in chunk:
                b = j * N_CORES + core
                vids = order[b * P : (b + 1) * P]  # [-1 for pad vertices]
                col = np.full(P, PADLOC, dtype=np.int64)
                real = vids >= 0
                v = vids[real]
                has = bnds[v, s] + c < bnds[v, s + 1]
                vv = v[has]
                col[np.where(real)[0][has]] = locs[vv, bnds[vv, s] + c]
                cols.append(col)
            lin = np.concatenate(cols)  # [ni], c-major (col block = 128)
            wrapped = lin.reshape(ni // 16, 16).T  # [16, ni/16]
            blocks.append(np.tile(wrapped, (P // 16, 1)).astype(np.int16))
        sup_block = np.concatenate(blocks, axis=1)  # [128, iw_sup]
        per_super_flat.append(sup_block.ravel())  # partition-major
    return np.concatenate(per_super_flat)


def build_nc(W, supers):
    import concourse.tile as tile
    from concourse import bacc, mybir

    iw_total = sum(ni // 16 for sup in supers for (_, _, ni) in sup["calls"])

    nc = bacc.Bacc("TRN2", target_bir_lowering=False, debug=False, num_swdge_queues=4)
    x_t = nc.dram_tensor("xaug", [VAUG, F], mybir.dt.float32, kind="ExternalInput")
    idx_t = nc.dram_tensor("idx16", [P * iw_total], mybir.dt.int16, kind="ExternalInput")
    out_t = nc.dram_tensor("out", [VS_PAD, 2 * F], mybir.dt.float32, kind="ExternalOutput")

    Ctot_max = max(sup["Ctot"] for sup in supers)
    iw_sup_max = max(
        sum(ni // 16 for (_, _, ni) in sup["calls"]) for sup in supers
    )

    with tile.TileContext(nc) as tc:
        with ExitStack() as ctx:
            idx_pool = ctx.enter_context(tc.tile_pool(name="idx", bufs=2))
            g_pool = ctx.enter_context(tc.tile_pool(name="gather", bufs=2))
            r_pool = ctx.enter_context(tc.tile_pool(name="red", bufs=2 * G))
            o_pool = ctx.enter_context(tc.tile_pool(name="out", bufs=2 * G))

            iw_off = 0
            for u, sup in enumerate(supers):
                iw_sup = sum(ni // 16 for (_, _, ni) in sup["calls"])
                idx_tile = idx_pool.tile([P, iw_sup_max], mybir.dt.int16, tag="idx")
                nc.sync.dma_start(
                    idx_tile[:, :iw_sup],
                    idx_t.ap()[iw_off * P : (iw_off + iw_sup) * P].rearrange(
                        "(p w) -> p w", p=P, w=iw_sup
                    ),
                )

                g_tile = g_pool.tile([P, Ctot_max * F], mybir.dt.float32, tag="g")
                col = 0
                ioff = 0
                for ci, (s, chunk, ni) in enumerate(sup["calls"]):
                    cw = ni // P
                    nc.gpsimd.dma_gather(
                        out_ap=g_tile[:, col * F : (col + cw) * F].rearrange(
                            "p (k f) -> p k f", k=cw, f=F
                        ),
                        in_ap=x_t.ap()[s * AUGB : (s + 1) * AUGB, :],
                        idxs_ap=idx_tile[:, ioff : ioff + ni // 16],
                        num_idxs=ni,
                        num_idxs_reg=ni,
                        elem_size=F,
                        queue_num=ci % 4,
                    )
                    col += cw
                    ioff += ni // 16
                iw_off += iw_sup

                for j in sup["tiles"]:
                    parts = [(s, sup["A"][(j, s)], int(W[j, s])) for s in range(NW)]
                    parts = [(s, a, w) for s, a, w in parts if w > 0]
                    if not parts:
                        continue  # all-dummy tile; rows are trimmed by the host
                    pm = []
                    ps = []
                    for s, a, w in parts:
                        ap3 = g_tile[:, a * F : (a + w) * F].rearrange(
                            "p (k f) -> p f k", k=w, f=F
                        )
                        if w == 1:
                            pm.append(g_tile[:, a * F : (a + 1) * F])
                            ps.append(g_tile[:, a * F : (a + 1) * F])
                            continue
                        tm = r_pool.tile([P, F], mybir.dt.float32, tag="rm")
                        ts_ = r_pool.tile([P, F], mybir.dt.float32, tag="rs")
                        nc.vector.tensor_reduce(
                            tm[:], ap3, axis=mybir.AxisListType.X,
                            op=mybir.AluOpType.max,
                        )
                        nc.vector.tensor_reduce(
                            ts_[:], ap3, axis=mybir.AxisListType.X,
                            op=mybir.AluOpType.add,
                        )
                        pm.append(tm[:])
                        ps.append(ts_[:])
                    while len(pm) > 1:
                        nc.vector.tensor_tensor(
                            out=pm[0], in0=pm[0], in1=pm[1],
                            op=mybir.AluOpType.max,
                        )
                        pm.pop(1)
                    while len(ps) > 1:
                        nc.vector.tensor_tensor(
                            out=ps[0], in0=ps[0], in1=ps[1],
                            op=mybir.AluOpType.add,
                        )
                        ps.pop(1)
                    o_tile = o_pool.tile([P, 2 * F], mybir.dt.float32, tag="o")
                    nc.scalar.activation(
                        o_tile[:, :F], ps[0], mybir.ActivationFunctionType.Copy,
                        bias=-BIAS, scale=1.0 / K,
                    )
                    nc.scalar.activation(
                        o_tile[:, F : 2 * F], pm[0],
                        mybir.ActivationFunctionType.Copy, bias=-BIAS,
                    )
                    nc.sync.dma_start(out_t.ap()[j * P : (j + 1) * P, :], o_tile[:])

    nc.compile()
    return nc


_CACHE = {}


def _get_compiled(idxs):
    key = hash(idxs.tobytes())
    if key not in _CACHE:
        order, W, locs, bnds = _plan(idxs)
        supers = _call_schedule(W)
        nc = build_nc(W, supers)
        _CACHE[key] = (nc, order, W, locs, bnds, supers)
    return _CACHE[key]


def _make_xaug(x):
    xa = np.zeros((VAUG, F), dtype=np.float32)
    for s in range(NW):
        xa[s * AUGB : s * AUGB + BLK] = x[s * BLK : (s + 1) * BLK] + BIAS
    return xa


def run(x, idxs, trace=False, trace_cores=None):
    from concourse.bass_utils import run_bass_kernel_spmd

    _install_ntff_hook()
    x = np.ascontiguousarray(np.asarray(x, dtype=np.float32))
    idxs = np.ascontiguousarray(np.asarray(idxs, dtype=np.int64))
    nc, order, W, locs, bnds, supers = _get_compiled(idxs)
    xaug = _make_xaug(x)

    in_maps = []
    for c in range(N_CORES):
        flat = _pack_idx16(c, order, W, locs, bnds, supers)
        in_maps.append({"xaug": xaug, "idx16": flat})

    res = run_bass_kernel_spmd(
        nc, in_maps, core_ids=list(range(N_CORES)), trace=trace,
        trace_cores=trace_cores,
    )

    out = np.empty((V, 2 * F), dtype=np.float32)
    for c in range(N_CORES):
        oc = res.results[c]["out"]  # [VS_PAD, 2F] in block order
        for t in range(NT):
            b = t * N_CORES + c
            vids = order[b * P : (b + 1) * P]
            real = vids >= 0
            out[vids[real]] = oc[t * P : (t + 1) * P][real]
    return out, res


def kernel(x, idxs):
    out, _ = run(x, idxs)
    return out



# revision 5
# speedup vs baseline: 1.7394x; 1.7394x over previous
"""Trainium2 kernel for CollectNeighbourAverageAndMax (gnn message passing).

out[v] = concat(mean_k x[idxs[v,k]], max_k x[idxs[v,k]]),  V=100000, K=32, F=64.

Sharding: vertices split across 8 NeuronCores (one SPMD program); x is
replicated per core so the irregular gather stays core-local.

Gather primitive: nc.gpsimd.dma_gather (int16 indices, 256B-stride window of
<=32768 rows). V=100000 rows cannot be addressed by int16, so NW=5
OVERLAPPING windows of 32768 rows cover the augmented array (a zero pad row
is inserted at each window start; real values carry +BIAS so pad rows never
win the max and add 0 to the sum). Each neighbour edge lies in 1-2 windows;
a per-vertex flow assignment balances every vertex's 32 edges toward
K/NW per window, which shrinks the per-tile padding (max count over the
1024 vertices sharing a tile row) from ~40% to ~12%.

Performance-critical structure (vs the naive version):
  * gather calls are spread round-robin over the 4 SWDGE queues -> the Q7
    descriptor generation runs on 4 cpu-pairs in parallel;
  * one gather tile per (super-tile, window) written by multiple calls with
    disjoint column ranges (parallel-safe);
  * calls carry up to 2048 indices (single_packet=False);
  * reductions are contiguous tensor_tensor fold trees (k-major halving),
    not strided tensor_reduce -- ~3x faster DVE reads, far fewer ops.
"""
import sys
import types
from contextlib import ExitStack

import numpy as np

V, K, F = 100000, 32, 64
N_CORES = 8
P = 128
NW = 5
WSPAN = 32768
VAUG = V + NW           # zero row inserted at each window start
BIAS = 16.0
NB = (V + P - 1) // P + (-((V + P - 1) // P)) % N_CORES  # 784 blocks (=8*98)
NT = NB // N_CORES      # 98 tiles per core
VS_PAD = NT * P         # 12544 padded vertices per core
G = 2                   # tiles per super-tile
NI_CAP = 2048           # max indices per dma_gather call (single_packet=False)

_STEP = (VAUG - WSPAN) / (NW - 1)
STARTS = np.round(np.arange(NW) * _STEP).astype(np.int64)
STARTS[-1] = VAUG - WSPAN


def _install_ntff_hook():
    try:
        import antenv

        if "antenv.axon_hooks" not in sys.modules:
            mod = types.ModuleType("antenv.axon_hooks")
            mod._hook = None
            mod.set_axon_ntff_profile_hook = lambda h: setattr(mod, "_hook", h)
            mod.get_axon_ntff_profile_hook = lambda: mod._hook
            sys.modules["antenv.axon_hooks"] = mod
            antenv.axon_hooks = mod
        if sys.modules["antenv.axon_hooks"]._hook is None:
            from trn_agent_boot.trn_boot import _ntff_profile_via_ctypes

            hook = _ntff_profile_via_ctypes("/opt/axon/libaxon_pjrt.so")
            sys.modules["antenv.axon_hooks"].set_axon_ntff_profile_hook(hook)
    except Exception:
        pass


def _plan(idxs):
    """Host-side plan.

    Returns (order [NB*P] vertex ids w/ -1 pad,
             Wsup [NSUP, NW] uniform per-super window widths,
             locs [V, K] window-local int16 values grouped by window,
             bnds [V, NW+1] group boundaries in locs,
             n [V, NW] per-vertex window counts)."""
    u = idxs.astype(np.int64)
    p = u.copy()
    for s in STARTS:
        p = p + (p >= s)
    # window membership: starts[s] <= p < starts[s]+WSPAN
    smax = np.searchsorted(STARTS, p.ravel(), "right").reshape(p.shape) - 1
    smin = np.searchsorted(STARTS + WSPAN, p.ravel(), "right").reshape(p.shape)
    assert (smin <= smax).all() and (smax - smin <= 1).all()

    e = np.stack([((smin == s) & (smax == s)).sum(1) for s in range(NW)], 1)
    f = np.stack([((smin == s) & (smax == s + 1)).sum(1) for s in range(NW - 1)], 1)
    # balance: start with all flex assigned right, relax toward equal counts
    n = e.copy()
    n[:, 1:] += f
    a = np.zeros((V, NW - 1), dtype=np.int64)  # flex(s,s+1) assigned LEFT
    for _ in range(16):
        moved = False
        for s in range(NW - 1):
            m = (n[:, s] < n[:, s + 1]) & (a[:, s] < f[:, s])
            if m.any():
                n[m, s] += 1
                n[m, s + 1] -= 1
                a[m, s] += 1
                moved = True
            m2 = (n[:, s] > n[:, s + 1] + 1) & (a[:, s] > 0)
            if m2.any():
                n[m2, s] -= 1
                n[m2, s + 1] += 1
                a[m2, s] -= 1
                moved = True
        if not moved:
            break
    assert (n.sum(1) == K).all()

    # per-edge window assignment consistent with (a, n)
    assigned = smin.copy()
    for s in range(NW - 1):
        m = (smin == s) & (smax == s + 1)
        rank = np.cumsum(m, axis=1) - 1
        go_right = m & (rank >= a[:, s : s + 1])
        assigned[go_right] = s + 1

    loc = p - STARTS[assigned]
    assert (loc > 0).all() and (loc < WSPAN).all()

    ordk = np.argsort(assigned, axis=1, kind="stable")
    locs = np.take_along_axis(loc, ordk, axis=1)
    bnds = np.zeros((V, NW + 1), dtype=np.int64)
    np.cumsum(n, axis=1, out=bnds[:, 1:])

    order = np.lexsort(tuple(n[:, c] for c in reversed(range(NW))))
    order = np.concatenate([order, np.full(NB * P - V, -1, dtype=np.int64)])

    NSUP = (NT + G - 1) // G
    n_pad = np.concatenate([n, np.zeros((1, NW), dtype=np.int64)])  # -1 -> 0
    grp = order.reshape(NT, N_CORES * P)
    Wt = n_pad[grp].max(axis=1)                    # [NT, NW] per-tile max
    Wsup = np.zeros((NSUP, NW), dtype=np.int64)
    for su in range(NSUP):
        Wsup[su] = Wt[su * G : (su + 1) * G].max(axis=0)
    return order, Wsup, locs, bnds, n


def _call_schedule(Wsup):
    """Per super-tile: for each window with W>0, chunk the G*W*P index stream
    into calls of <= NI_CAP indices. Returns list over supers of dicts."""
    NSUP = Wsup.shape[0]
    supers = []
    for su in range(NSUP):
        tiles = list(range(su * G, min((su + 1) * G, NT)))
        gl = len(tiles)
        wins = []
        for s in range(NW):
            W = int(Wsup[su, s])
            if W == 0:
                continue
            ncols = gl * W
            calls = []
            c0 = 0
            maxcols = NI_CAP // P
            while c0 < ncols:
                cw = min(maxcols, ncols - c0)
                calls.append((c0, cw))
                c0 += cw
            wins.append({"s": s, "W": W, "calls": calls, "ncols": ncols})
        supers.append({"su": su, "tiles": tiles, "gl": gl, "wins": wins})
    return supers


def _pack_idx16(core, order, supers, locs, bnds, n):
    """Flat int16 index stream for this core. Column order per (super, win):
    t-major then k (col = t_local*W + k). Each call's [128, ni] block is
    wrapped to [16, ni/16], replicated to [128, ni/16]; calls and supers are
    concatenated along the free dim; the whole stream is raveled
    partition-major."""
    out = []
    for sup in supers:
        blocks = []
        for win in sup["wins"]:
            s, W = win["s"], win["W"]
            # per-column [P] indices for all ncols, vectorized per tile
            cols = np.zeros((win["ncols"], P), dtype=np.int64)
            for tl, j in enumerate(sup["tiles"]):
                b = j * N_CORES + core
                vids = order[b * P : (b + 1) * P]
                real = vids >= 0
                v = vids[real]
                base = bnds[v, s]
                cnt = n[v, s]
                for k in range(W):
                    col = np.zeros(P, dtype=np.int64)
                    has = k < cnt
                    col[np.where(real)[0][has]] = locs[v[has], base[has] + k]
                    cols[tl * W + k] = col
            for c0, cw in win["calls"]:
                lin = cols[c0 : c0 + cw].ravel()  # [cw*P] column-major blocks
                ni = cw * P
                wrapped = lin.reshape(ni // 16, 16).T
                blocks.append(np.tile(wrapped, (P // 16, 1)).astype(np.int16))
        sup_block = np.concatenate(blocks, axis=1) if blocks else None
        if sup_block is not None:
            out.append(sup_block.ravel())
    return np.concatenate(out)


def build_nc(supers):
    import concourse.tile as tile
    from concourse import bacc, mybir

    iw_total = sum(
        (cw * P) // 16 for sup in supers for win in sup["wins"] for (_, cw) in win["calls"]
    )

    nc = bacc.Bacc("TRN2", target_bir_lowering=False, debug=False, num_swdge_queues=4)
    x_t = nc.dram_tensor("xaug", [VAUG, F], mybir.dt.float32, kind="ExternalInput")
    idx_t = nc.dram_tensor("idx16", [P * iw_total], mybir.dt.int16, kind="ExternalInput")
    out_t = nc.dram_tensor("out", [VS_PAD, 2 * F], mybir.dt.float32, kind="ExternalOutput")

    iw_sup_max = max(
        sum((cw * P) // 16 for win in sup["wins"] for (_, cw) in win["calls"])
        for sup in supers
    )

    with tile.TileContext(nc) as tc:
        with ExitStack() as ctx:
            idx_pool = ctx.enter_context(tc.tile_pool(name="idx", bufs=2))
            g_pool = ctx.enter_context(tc.tile_pool(name="gather", bufs=2))
            sc_pool = ctx.enter_context(tc.tile_pool(name="scr", bufs=2))
            cb_pool = ctx.enter_context(tc.tile_pool(name="comb", bufs=2))
            o_pool = ctx.enter_context(tc.tile_pool(name="out", bufs=2))

            iw_off = 0
            rr = 0  # global queue round-robin
            for sup in supers:
                gl = len(sup["tiles"])
                iw_sup = sum(
                    (cw * P) // 16 for win in sup["wins"] for (_, cw) in win["calls"]
                )
                idx_tile = idx_pool.tile([P, iw_sup_max], mybir.dt.int16, tag="idx")
                nc.sync.dma_start(
                    idx_tile[:, :iw_sup],
                    idx_t.ap()[iw_off * P : (iw_off + iw_sup) * P].rearrange(
                        "(p w) -> p w", p=P, w=iw_sup
                    ),
                )
                iw_off += iw_sup

                ioff = 0
                gtiles = {}
                for win in sup["wins"]:
                    s, W, ncols = win["s"], win["W"], win["ncols"]
                    g_tile = g_pool.tile([P, ncols * F], mybir.dt.float32, tag=f"g{s}")
                    gtiles[s] = g_tile
                    for c0, cw in win["calls"]:
                        ni = cw * P
                        nc.gpsimd.dma_gather(
                            out_ap=g_tile[:, c0 * F : (c0 + cw) * F].rearrange(
                                "p (k f) -> p k f", k=cw, f=F
                            ),
                            in_ap=x_t.ap()[STARTS[s] : STARTS[s] + WSPAN, :],
                            idxs_ap=idx_tile[:, ioff : ioff + ni // 16],
                            num_idxs=ni,
                            num_idxs_reg=ni,
                            elem_size=F,
                            queue_num=rr % 4,
                            single_packet=False,
                        )
                        rr += 1
                        ioff += ni // 16

                # reductions: per (window): fold trees over k (t-major layout)
                pm = []  # (view [p, gl, 1, f]) max partials per window
                ps = []
                for win in sup["wins"]:
                    s, W = win["s"], win["W"]
                    g_tile = gtiles[s]
                    gv = lambda k0, k1, W=W, g_tile=g_tile: g_tile[
                        :, : gl * W * F
                    ].rearrange("p (t k f) -> p t k f", t=gl, k=W, f=F)[
                        :, :, k0:k1, :
                    ]
                    if W == 1:
                        pm.append(gv(0, 1))
                        ps.append(gv(0, 1))
                        continue
                    # sum: first fold g -> scratch (+ survivor copy), then in place
                    m = W // 2
                    h = W - m
                    scr = sc_pool.tile([P, gl * h * F], mybir.dt.float32, tag=f"s{s}")
                    sv = lambda k0, k1, h=h, scr=scr: scr[:].rearrange(
                        "p (t k f) -> p t k f", t=gl, k=h, f=F
                    )[:, :, k0:k1, :]
                    nc.vector.tensor_tensor(
                        out=sv(0, m), in0=gv(0, m), in1=gv(h, W),
                        op=mybir.AluOpType.add,
                    )
                    if h > m:  # odd W: copy the survivor column block
                        nc.vector.tensor_tensor(
                            out=sv(m, h), in0=gv(m, h), in1=gv(m, h),
                            op=mybir.AluOpType.bypass,
                        )
                    T = h
                    while T > 1:
                        m2 = T // 2
                        h2 = T - m2
                        nc.vector.tensor_tensor(
                            out=sv(0, m2), in0=sv(0, m2), in1=sv(h2, T),
                            op=mybir.AluOpType.add,
                        )
                        T = h2
                    ps.append(sv(0, 1))
                    # max: in-place fold in g
                    T = W
                    while T > 1:
                        m2 = T // 2
                        h2 = T - m2
                        nc.vector.tensor_tensor(
                            out=gv(0, m2), in0=gv(0, m2), in1=gv(h2, T),
                            op=mybir.AluOpType.max,
                        )
                        T = h2
                    pm.append(gv(0, 1))

                # combine across windows -> comb tiles [p, gl*F]
                cm = cb_pool.tile([P, gl * F], mybir.dt.float32, tag="cm")
                cs = cb_pool.tile([P, gl * F], mybir.dt.float32, tag="cs")
                cmv = cm[:].rearrange("p (t k f) -> p t k f", t=gl, k=1, f=F)
                csv = cs[:].rearrange("p (t k f) -> p t k f", t=gl, k=1, f=F)
                nc.vector.tensor_tensor(
                    out=cmv, in0=pm[0], in1=pm[1] if len(pm) > 1 else pm[0],
                    op=mybir.AluOpType.max,
                )
                for q in pm[2:]:
                    nc.vector.tensor_tensor(out=cmv, in0=cmv, in1=q,
                                            op=mybir.AluOpType.max)
                nc.vector.tensor_tensor(
                    out=csv, in0=ps[0], in1=ps[1] if len(ps) > 1 else None
                    or ps[0], op=mybir.AluOpType.add if len(ps) > 1 else
                    mybir.AluOpType.bypass,
                )
                for q in ps[2:]:
                    nc.vector.tensor_tensor(out=csv, in0=csv, in1=q,
                                            op=mybir.AluOpType.add)

                # finalize + store
                o_tile = o_pool.tile([P, gl * 2 * F], mybir.dt.float32, tag="o")
                for tl in range(gl):
                    nc.scalar.activation(
                        o_tile[:, tl * 2 * F : tl * 2 * F + F],
                        cs[:, tl * F : (tl + 1) * F],
                        mybir.ActivationFunctionType.Copy,
                        bias=-BIAS, scale=1.0 / K,
                    )
                    nc.scalar.activation(
                        o_tile[:, tl * 2 * F + F : (tl + 1) * 2 * F],
                        cm[:, tl * F : (tl + 1) * F],
                        mybir.ActivationFunctionType.Copy, bias=-BIAS,
                    )
                j0 = sup["tiles"][0]
                nc.sync.dma_start(
                    out_t.ap()[j0 * P : (j0 + gl) * P, :].rearrange(
                        "(t p) c -> p t c", t=gl, p=P
                    ),
                    o_tile[:].rearrange("p (t c) -> p t c", t=gl, c=2 * F),
                )

    nc.compile()
    return nc


_CACHE = {}


def _get_compiled(idxs):
    key = hash(idxs.tobytes())
    if key not in _CACHE:
        order, Wsup, locs, bnds, n = _plan(idxs)
        supers = _call_schedule(Wsup)
        nc = build_nc(supers)
        _CACHE[key] = (nc, order, supers, locs, bnds, n)
    return _CACHE[key]


def _make_xaug(x):
    xa = np.zeros((VAUG, F), dtype=np.float32)
    pos = np.arange(V, dtype=np.int64)
    for s in STARTS:
        pos = pos + (pos >= s)
    xa[pos] = x + BIAS
    return xa


def run(x, idxs, trace=False, trace_cores=None):
    from concourse.bass_utils import run_bass_kernel_spmd

    _install_ntff_hook()
    x = np.ascontiguousarray(np.asarray(x, dtype=np.float32))
    idxs = np.ascontiguousarray(np.asarray(idxs, dtype=np.int64))
    nc, order, supers, locs, bnds, n = _get_compiled(idxs)
    xaug = _make_xaug(x)

    in_maps = []
    for c in range(N_CORES):
        flat = _pack_idx16(c, order, supers, locs, bnds, n)
        in_maps.append({"xaug": xaug, "idx16": flat})

    res = run_bass_kernel_spmd(
        nc, in_maps, core_ids=list(range(N_CORES)), trace=trace,
        trace_cores=trace_cores,
    )

    out = np.empty((V, 2 * F), dtype=np.float32)
    for c in range(N_CORES):
        oc = res.results[c]["out"]  # [VS_PAD, 2F] in block order
        for t in range(NT):
            b = t * N_CORES + c
            vids = order[b * P : (b + 1) * P]
            real = vids >= 0
            out[vids[real]] = oc[t * P : (t + 1) * P][real]
    return out, res


def kernel(x, idxs):
    out, _ = run(x, idxs)
    return out


# revision 6
# speedup vs baseline: 1.8321x; 1.0533x over previous
"""Trainium2 kernel for CollectNeighbourAverageAndMax (gnn message passing).

out[v] = concat(mean_k x[idxs[v,k]], max_k x[idxs[v,k]]),  V=100000, K=32, F=64.

Sharding: vertices split across 8 NeuronCores (one SPMD program); x is
replicated per core so the irregular gather stays core-local.

Gather primitive: nc.gpsimd.dma_gather (int16 indices, 256B-stride window of
<=32768 rows). V=100000 rows cannot be addressed by int16, so NW=5
OVERLAPPING windows of 32768 rows cover the augmented array (a zero pad row
is inserted at each window start; real values carry +BIAS so pad rows never
win the max and add 0 to the sum). Each neighbour edge lies in 1-2 windows;
a per-vertex flow assignment balances every vertex's 32 edges toward
K/NW per window, which shrinks the per-tile padding (max count over the
1024 vertices sharing a tile row) from ~40% to ~12%.

Performance-critical structure (vs the naive version):
  * gather calls are spread round-robin over the 4 SWDGE queues -> the Q7
    descriptor generation runs on 4 cpu-pairs in parallel;
  * one gather tile per (super-tile, window) written by multiple calls with
    disjoint column ranges (parallel-safe);
  * calls carry up to 2048 indices (single_packet=False);
  * reductions are contiguous tensor_tensor fold trees (k-major halving),
    not strided tensor_reduce -- ~3x faster DVE reads, far fewer ops.
"""
import sys
import types
from contextlib import ExitStack

import numpy as np

V, K, F = 100000, 32, 64
N_CORES = 8
P = 128
NW = 5
WSPAN = 32768
VAUG = V + NW           # zero row inserted at each window start
BIAS = 16.0
NB = (V + P - 1) // P + (-((V + P - 1) // P)) % N_CORES  # 784 blocks (=8*98)
NT = NB // N_CORES      # 98 tiles per core
VS_PAD = NT * P         # 12544 padded vertices per core
G = 2                   # tiles per super-tile
NI_CAP = 2048           # max indices per dma_gather call (single_packet=False)

_STEP = (VAUG - WSPAN) / (NW - 1)
STARTS = np.round(np.arange(NW) * _STEP).astype(np.int64)
STARTS[-1] = VAUG - WSPAN


def _install_ntff_hook():
    try:
        import antenv

        if "antenv.axon_hooks" not in sys.modules:
            mod = types.ModuleType("antenv.axon_hooks")
            mod._hook = None
            mod.set_axon_ntff_profile_hook = lambda h: setattr(mod, "_hook", h)
            mod.get_axon_ntff_profile_hook = lambda: mod._hook
            sys.modules["antenv.axon_hooks"] = mod
            antenv.axon_hooks = mod
        if sys.modules["antenv.axon_hooks"]._hook is None:
            from trn_agent_boot.trn_boot import _ntff_profile_via_ctypes

            hook = _ntff_profile_via_ctypes("/opt/axon/libaxon_pjrt.so")
            sys.modules["antenv.axon_hooks"].set_axon_ntff_profile_hook(hook)
    except Exception:
        pass


def _plan(idxs):
    """Host-side plan.

    Returns (order [NB*P] vertex ids w/ -1 pad,
             Wsup [NSUP, NW] uniform per-super window widths,
             locs [V, K] window-local int16 values grouped by window,
             bnds [V, NW+1] group boundaries in locs,
             n [V, NW] per-vertex window counts)."""
    u = idxs.astype(np.int64)
    p = u.copy()
    for s in STARTS:
        p = p + (p >= s)
    # window membership: starts[s] <= p < starts[s]+WSPAN
    smax = np.searchsorted(STARTS, p.ravel(), "right").reshape(p.shape) - 1
    smin = np.searchsorted(STARTS + WSPAN, p.ravel(), "right").reshape(p.shape)
    assert (smin <= smax).all() and (smax - smin <= 1).all()

    e = np.stack([((smin == s) & (smax == s)).sum(1) for s in range(NW)], 1)
    f = np.stack([((smin == s) & (smax == s + 1)).sum(1) for s in range(NW - 1)], 1)
    # balance: start with all flex assigned right, relax toward equal counts
    n = e.copy()
    n[:, 1:] += f
    a = np.zeros((V, NW - 1), dtype=np.int64)  # flex(s,s+1) assigned LEFT
    for _ in range(16):
        moved = False
        for s in range(NW - 1):
            m = (n[:, s] < n[:, s + 1]) & (a[:, s] < f[:, s])
            if m.any():
                n[m, s] += 1
                n[m, s + 1] -= 1
                a[m, s] += 1
                moved = True
            m2 = (n[:, s] > n[:, s + 1] + 1) & (a[:, s] > 0)
            if m2.any():
                n[m2, s] -= 1
                n[m2, s + 1] += 1
                a[m2, s] -= 1
                moved = True
        if not moved:
            break
    assert (n.sum(1) == K).all()

    # per-edge window assignment consistent with (a, n)
    assigned = smin.copy()
    for s in range(NW - 1):
        m = (smin == s) & (smax == s + 1)
        rank = np.cumsum(m, axis=1) - 1
        go_right = m & (rank >= a[:, s : s + 1])
        assigned[go_right] = s + 1

    loc = p - STARTS[assigned]
    assert (loc > 0).all() and (loc < WSPAN).all()

    ordk = np.argsort(assigned, axis=1, kind="stable")
    locs = np.take_along_axis(loc, ordk, axis=1)
    bnds = np.zeros((V, NW + 1), dtype=np.int64)
    np.cumsum(n, axis=1, out=bnds[:, 1:])

    order = np.lexsort(tuple(n[:, c] for c in reversed(range(NW))))
    order = np.concatenate([order, np.full(NB * P - V, -1, dtype=np.int64)])

    NSUP = (NT + G - 1) // G
    n_pad = np.concatenate([n, np.zeros((1, NW), dtype=np.int64)])  # -1 -> 0
    grp = order.reshape(NT, N_CORES * P)
    Wt = n_pad[grp].max(axis=1)                    # [NT, NW] per-tile max
    Wsup = np.zeros((NSUP, NW), dtype=np.int64)
    for su in range(NSUP):
        Wsup[su] = Wt[su * G : (su + 1) * G].max(axis=0)
    return order, Wsup, locs, bnds, n


def _call_schedule(Wsup):
    """Per super-tile: for each window with W>0, chunk the G*W*P index stream
    into calls of <= NI_CAP indices. Returns list over supers of dicts."""
    NSUP = Wsup.shape[0]
    supers = []
    for su in range(NSUP):
        tiles = list(range(su * G, min((su + 1) * G, NT)))
        gl = len(tiles)
        wins = []
        for s in range(NW):
            W = int(Wsup[su, s])
            if W == 0:
                continue
            ncols = gl * W
            calls = []
            c0 = 0
            maxcols = NI_CAP // P
            while c0 < ncols:
                cw = min(maxcols, ncols - c0)
                calls.append((c0, cw))
                c0 += cw
            wins.append({"s": s, "W": W, "calls": calls, "ncols": ncols})
        supers.append({"su": su, "tiles": tiles, "gl": gl, "wins": wins})
    return supers


def _pack_idx16(core, order, supers, locs, bnds, n):
    """Flat int16 index stream for this core. Column order per (super, win):
    t-major then k (col = t_local*W + k). Each call's [128, ni] block is
    wrapped to [16, ni/16], replicated to [128, ni/16]; calls and supers are
    concatenated along the free dim; the whole stream is raveled
    partition-major."""
    out = []
    for sup in supers:
        blocks = []
        for win in sup["wins"]:
            s, W = win["s"], win["W"]
            # per-column [P] indices for all ncols, vectorized per tile
            cols = np.zeros((win["ncols"], P), dtype=np.int64)
            for tl, j in enumerate(sup["tiles"]):
                b = j * N_CORES + core
                vids = order[b * P : (b + 1) * P]
                real = vids >= 0
                v = vids[real]
                base = bnds[v, s]
                cnt = n[v, s]
                for k in range(W):
                    col = np.zeros(P, dtype=np.int64)
                    has = k < cnt
                    col[np.where(real)[0][has]] = locs[v[has], base[has] + k]
                    cols[tl * W + k] = col
            for c0, cw in win["calls"]:
                lin = cols[c0 : c0 + cw].ravel()  # [cw*P] column-major blocks
                ni = cw * P
                wrapped = lin.reshape(ni // 16, 16).T
                blocks.append(np.tile(wrapped, (P // 16, 1)).astype(np.int16))
        sup_block = np.concatenate(blocks, axis=1) if blocks else None
        if sup_block is not None:
            out.append(sup_block.ravel())
    return np.concatenate(out)


def build_nc(supers):
    import concourse.tile as tile
    from concourse import bacc, mybir

    iw_total = sum(
        (cw * P) // 16 for sup in supers for win in sup["wins"] for (_, cw) in win["calls"]
    )

    nc = bacc.Bacc("TRN2", target_bir_lowering=False, debug=False, num_swdge_queues=4)
    x_t = nc.dram_tensor("xaug", [VAUG, F], mybir.dt.float32, kind="ExternalInput")
    idx_t = nc.dram_tensor("idx16", [P * iw_total], mybir.dt.int16, kind="ExternalInput")
    out_t = nc.dram_tensor("out", [VS_PAD, 2 * F], mybir.dt.float32, kind="ExternalOutput")

    iw_sup_max = max(
        sum((cw * P) // 16 for win in sup["wins"] for (_, cw) in win["calls"])
        for sup in supers
    )

    with tile.TileContext(nc) as tc:
        with ExitStack() as ctx:
            idx_pool = ctx.enter_context(tc.tile_pool(name="idx", bufs=4))
            g_pool = ctx.enter_context(tc.tile_pool(name="gather", bufs=3))
            sc_pool = ctx.enter_context(tc.tile_pool(name="scr", bufs=3))
            cb_pool = ctx.enter_context(tc.tile_pool(name="comb", bufs=3))
            o_pool = ctx.enter_context(tc.tile_pool(name="out", bufs=3))

            iw_off = 0
            rr = 0  # global queue round-robin
            for sup in supers:
                gl = len(sup["tiles"])
                iw_sup = sum(
                    (cw * P) // 16 for win in sup["wins"] for (_, cw) in win["calls"]
                )
                idx_tile = idx_pool.tile([P, iw_sup_max], mybir.dt.int16, tag="idx")
                nc.sync.dma_start(
                    idx_tile[:, :iw_sup],
                    idx_t.ap()[iw_off * P : (iw_off + iw_sup) * P].rearrange(
                        "(p w) -> p w", p=P, w=iw_sup
                    ),
                )
                iw_off += iw_sup

                ioff = 0
                gtiles = {}
                for win in sup["wins"]:
                    s, W, ncols = win["s"], win["W"], win["ncols"]
                    g_tile = g_pool.tile([P, ncols * F], mybir.dt.float32, tag=f"g{s}")
                    gtiles[s] = g_tile
                    for c0, cw in win["calls"]:
                        ni = cw * P
                        nc.gpsimd.dma_gather(
                            out_ap=g_tile[:, c0 * F : (c0 + cw) * F].rearrange(
                                "p (k f) -> p k f", k=cw, f=F
                            ),
                            in_ap=x_t.ap()[STARTS[s] : STARTS[s] + WSPAN, :],
                            idxs_ap=idx_tile[:, ioff : ioff + ni // 16],
                            num_idxs=ni,
                            num_idxs_reg=ni,
                            elem_size=F,
                            queue_num=rr % 4,
                            single_packet=False,
                        )
                        rr += 1
                        ioff += ni // 16

                # reductions: per (window): fold trees over k (t-major layout)
                pm = []  # (view [p, gl, 1, f]) max partials per window
                ps = []
                for win in sup["wins"]:
                    s, W = win["s"], win["W"]
                    g_tile = gtiles[s]
                    gv = lambda k0, k1, W=W, g_tile=g_tile: g_tile[
                        :, : gl * W * F
                    ].rearrange("p (t k f) -> p t k f", t=gl, k=W, f=F)[
                        :, :, k0:k1, :
                    ]
                    if W == 1:
                        pm.append(gv(0, 1))
                        ps.append(gv(0, 1))
                        continue
                    # sum: first fold g -> scratch (+ survivor copy), then in place
                    m = W // 2
                    h = W - m
                    scr = sc_pool.tile([P, gl * h * F], mybir.dt.float32, tag=f"s{s}")
                    sv = lambda k0, k1, h=h, scr=scr: scr[:].rearrange(
                        "p (t k f) -> p t k f", t=gl, k=h, f=F
                    )[:, :, k0:k1, :]
                    nc.vector.tensor_tensor(
                        out=sv(0, m), in0=gv(0, m), in1=gv(h, W),
                        op=mybir.AluOpType.add,
                    )
                    if h > m:  # odd W: copy the survivor column block
                        nc.vector.tensor_tensor(
                            out=sv(m, h), in0=gv(m, h), in1=gv(m, h),
                            op=mybir.AluOpType.bypass,
                        )
                    T = h
                    while T > 1:
                        m2 = T // 2
                        h2 = T - m2
                        nc.vector.tensor_tensor(
                            out=sv(0, m2), in0=sv(0, m2), in1=sv(h2, T),
                            op=mybir.AluOpType.add,
                        )
                        T = h2
                    ps.append(sv(0, 1))
                    # max: in-place fold in g
                    T = W
                    while T > 1:
                        m2 = T // 2
                        h2 = T - m2
                        nc.vector.tensor_tensor(
                            out=gv(0, m2), in0=gv(0, m2), in1=gv(h2, T),
                            op=mybir.AluOpType.max,
                        )
                        T = h2
                    pm.append(gv(0, 1))

                # combine across windows -> comb tiles [p, gl*F]
                cm = cb_pool.tile([P, gl * F], mybir.dt.float32, tag="cm")
                cs = cb_pool.tile([P, gl * F], mybir.dt.float32, tag="cs")
                cmv = cm[:].rearrange("p (t k f) -> p t k f", t=gl, k=1, f=F)
                csv = cs[:].rearrange("p (t k f) -> p t k f", t=gl, k=1, f=F)
                nc.vector.tensor_tensor(
                    out=cmv, in0=pm[0], in1=pm[1] if len(pm) > 1 else pm[0],
                    op=mybir.AluOpType.max,
                )
                for q in pm[2:]:
                    nc.vector.tensor_tensor(out=cmv, in0=cmv, in1=q,
                                            op=mybir.AluOpType.max)
                nc.vector.tensor_tensor(
                    out=csv, in0=ps[0], in1=ps[1] if len(ps) > 1 else None
                    or ps[0], op=mybir.AluOpType.add if len(ps) > 1 else
                    mybir.AluOpType.bypass,
                )
                for q in ps[2:]:
                    nc.vector.tensor_tensor(out=csv, in0=csv, in1=q,
                                            op=mybir.AluOpType.add)

                # finalize + store
                o_tile = o_pool.tile([P, gl * 2 * F], mybir.dt.float32, tag="o")
                for tl in range(gl):
                    nc.scalar.activation(
                        o_tile[:, tl * 2 * F : tl * 2 * F + F],
                        cs[:, tl * F : (tl + 1) * F],
                        mybir.ActivationFunctionType.Copy,
                        bias=-BIAS, scale=1.0 / K,
                    )
                    nc.scalar.activation(
                        o_tile[:, tl * 2 * F + F : (tl + 1) * 2 * F],
                        cm[:, tl * F : (tl + 1) * F],
                        mybir.ActivationFunctionType.Copy, bias=-BIAS,
                    )
                j0 = sup["tiles"][0]
                nc.sync.dma_start(
                    out_t.ap()[j0 * P : (j0 + gl) * P, :].rearrange(
                        "(t p) c -> p t c", t=gl, p=P
                    ),
                    o_tile[:].rearrange("p (t c) -> p t c", t=gl, c=2 * F),
                )

    nc.compile()
    return nc


_CACHE = {}


def _get_compiled(idxs):
    key = hash(idxs.tobytes())
    if key not in _CACHE:
        order, Wsup, locs, bnds, n = _plan(idxs)
        supers = _call_schedule(Wsup)
        nc = build_nc(supers)
        _CACHE[key] = (nc, order, supers, locs, bnds, n)
    return _CACHE[key]


def _make_xaug(x):
    xa = np.zeros((VAUG, F), dtype=np.float32)
    pos = np.arange(V, dtype=np.int64)
    for s in STARTS:
        pos = pos + (pos >= s)
    xa[pos] = x + BIAS
    return xa


def run(x, idxs, trace=False, trace_cores=None):
    from concourse.bass_utils import run_bass_kernel_spmd

    _install_ntff_hook()
    x = np.ascontiguousarray(np.asarray(x, dtype=np.float32))
    idxs = np.ascontiguousarray(np.asarray(idxs, dtype=np.int64))
    nc, order, supers, locs, bnds, n = _get_compiled(idxs)
    xaug = _make_xaug(x)

    in_maps = []
    for c in range(N_CORES):
        flat = _pack_idx16(c, order, supers, locs, bnds, n)
        in_maps.append({"xaug": xaug, "idx16": flat})

    res = run_bass_kernel_spmd(
        nc, in_maps, core_ids=list(range(N_CORES)), trace=trace,
        trace_cores=trace_cores,
    )

    out = np.empty((V, 2 * F), dtype=np.float32)
    for c in range(N_CORES):
        oc = res.results[c]["out"]  # [VS_PAD, 2F] in block order
        for t in range(NT):
            b = t * N_CORES + c
            vids = order[b * P : (b + 1) * P]
            real = vids >= 0
            out[vids[real]] = oc[t * P : (t + 1) * P][real]
    return out, res


def kernel(x, idxs):
    out, _ = run(x, idxs)
    return out
